# revision 11
# baseline (speedup 1.0000x reference)
"""ClusterMergeNet Trainium2 kernel, v2: triangle-hugging tiling.

Math: sim[b,i,j] (i<j) = sigmoid( sum_h W2c[h]*relu(A[b,i,h] + C[b,j,h] + b1c[h]) + b2c )
  with A = X @ W1c[:D], C = X @ W1c[D:]   (pair-MLP first layer decomposed).
Diagonal: sigmoid( sum_h W2s[h]*relu((X@W1s)[b,i,h] + b1s[h]) + b2s ).

|W2c| is folded into the first-layer weights/bias on the host so the device
reduction over h is a signed sum via one-hot sign-column matmuls on the PE.

Tiling (v2): per batch b, rows are split into 16 groups of 32 (group g =
rows [32g, 32g+32)); the upper triangle only needs j >= 32g for group g, so
the j-span is [32g, 512) (width 512-32g).  Core role r in {0,1} (cores 2b+r)
takes the 16-row half [32g+16r, 32g+16(r+1)) of every group -> all 8 cores
run an IDENTICAL instruction stream (SPMD); only the packed xit data differs.
Work per core: 16 groups x 16 rows x (512-32g) cols = 69632 streamed columns
vs 98304 in v1 (-29%).

Generation m = relu(C_tile + a_i) is split over three engines (DVE fastest
with the 2-byte 4x mode, then ACT, then Pool at 0.6 efficiency); the PE
consumes m tiles with sliding sign-window matmuls accumulating 16 rows into
one PSUM quadrant per group; ACT applies sigmoid and DMAs [16, w] bf16 tiles.

The walrus build on this image supports only ~1 semaphore wait per hardware
instruction; same mitigation as v1 (packed input DMAs + per-engine fan-in
NOPs, per-engine tile pools, NOP-guarded output DMAs, _split_waits postpass).
"""

import threading

import numpy as np
import ml_dtypes

B, N, D, H = 4, 512, 128, 128
NCORES = 8
NG = 16                      # groups per core
WIDTHS = [512 - 32 * g for g in range(NG)]

# bf16 pack layout (columns)
C_XT = 0            # [0,512)    X_b.T
C_XIT = 512         # [512,768)  own 256 i-rows (processing order), transposed
C_W1A = 768         # [768,896)  W1c[:D]*|W2c|
C_W1B = 896         # [896,1024) W1c[D:]*|W2c|
C_W1S = 1024        # [1024,1152) W1s*|W2s|
C_ZZ = 1152         # [1152,1216) zz (col C_ZZ+32 = sign(W2c))
C_ZS = 1216         # sign(W2s)
PK16_W = 1217
# fp32 pack: beffc, b1se, b2c, b2s
PK32_W = 4

# engine per within-group position k: 10x DVE, 2x ACT, 4x Pool (interleaved)
GEN_ENG = ['D', 'D', 'P', 'D', 'A', 'D', 'P', 'D',
           'D', 'D', 'P', 'D', 'A', 'D', 'D', 'D']

_lock = threading.Lock()
_cache = {}


def _build_nc():
    import concourse.bass as bass
    import concourse.mybir as mybir
    import concourse.tile as tile
    from concourse.tile import add_dep_helper

    fp32 = mybir.dt.float32
    bf16 = mybir.dt.bfloat16

    nc = bass.Bass("TRN2")
    pk32 = nc.dram_tensor("pk32", [128, PK32_W], fp32, kind="ExternalInput")
    pk16 = nc.dram_tensor("pk16", [128, PK16_W], bf16, kind="ExternalInput")
    QW = [sum(WIDTHS[c + 4 * t] for t in range(4)) for c in range(4)]
    outs = [nc.dram_tensor(f"q{c}", [16, QW[c]], bf16, kind="ExternalOutput")
            for c in range(4)]
    dd = nc.dram_tensor("dd", [1, 256], fp32, kind="ExternalOutput")

    exit_prods = []

    def spnop(prod):
        n = nc.sync.nop(nofuse=True)
        add_dep_helper(n.ins, prod.ins, sync=True, reason="sp fanin")
        return n

    with tile.TileContext(nc) as tc:
        with (
            tc.tile_pool(name="singles", bufs=1) as singles,
            tc.tile_pool(name="mdve", bufs=16) as mdve,
            tc.tile_pool(name="mact", bufs=4) as mact,
            tc.tile_pool(name="mpool", bufs=8) as mpool,
            tc.tile_pool(name="vout", bufs=1) as vout,
            tc.tile_pool(name="pre_ps", bufs=1, space="PSUM") as pre_ps,
            tc.tile_pool(name="acc_ps", bufs=6, space="PSUM") as acc_ps,
        ):
            p16 = singles.tile([128, PK16_W], bf16, tag="p16")
            dma16 = nc.sync.dma_start(out=p16, in_=pk16[:, :])
            p32 = singles.tile([128, PK32_W], fp32, tag="p32")
            dma32 = nc.sync.dma_start(out=p32, in_=pk32[:, :])

            xt = p16[:, C_XT:C_XT + 512]
            xit = p16[:, C_XIT:C_XIT + 256]
            w1a = p16[:, C_W1A:C_W1A + 128]
            w1b = p16[:, C_W1B:C_W1B + 128]
            w1s = p16[:, C_W1S:C_W1S + 128]
            zz = p16[:, C_ZZ:C_ZZ + 64]
            zs = p16[:, C_ZS:C_ZS + 1]
            beffc = p32[:, 0:1]
            b1se = p32[:, 1:2]
            b2c = p32[:, 2:3]
            b2s = p32[:, 3:4]

            # --- precompute: abias = (w1a.T @ xit) + beffc   [H, 256] f32 ---
            apsum = pre_ps.tile([H, 256], fp32, tag="pre")
            nc.tensor.matmul(apsum, w1a, xit, start=True, stop=True)
            abias = singles.tile([H, 256], fp32, tag="abias")
            nc.vector.tensor_scalar(out=abias, in0=apsum, scalar1=beffc,
                                    scalar2=None, op0=mybir.AluOpType.add)

            # --- c2 = (w1b.T @ xt)  [H, N] bf16 (bias lives in abias) ---
            cpsum = pre_ps.tile([H, N], fp32, tag="cpsum")
            nc.tensor.matmul(cpsum, w1b, xt, start=True, stop=True)
            c2 = singles.tile([H, N], bf16, tag="c2")
            nc.vector.tensor_copy(out=c2, in_=cpsum)

            spsum = pre_ps.tile([H, 256], fp32, tag="pre", name="spsum")
            nc.tensor.matmul(spsum, w1s, xit, start=True, stop=True)

            def emit_diag():
                # rest of the self-sim diagonal (emitted after group 0 so it
                # stays off the prologue critical path)
                ms = singles.tile([H, 256], bf16, tag="ms")
                nc.vector.tensor_scalar(out=ms, in0=spsum, scalar1=b1se,
                                        scalar2=0.0, op0=mybir.AluOpType.add,
                                        op1=mybir.AluOpType.max)
                dpsum = pre_ps.tile([128, 256], fp32, tag="pre", name="dpsum")
                nc.tensor.matmul(dpsum[0:1, :], zs, ms, start=True, stop=True)
                dsig = singles.tile([1, 256], fp32, tag="dsig")
                act_d = nc.scalar.activation(
                    out=dsig, in_=dpsum[0:1, :],
                    func=mybir.ActivationFunctionType.Sigmoid,
                    bias=b2s[0:1, :], scale=1.0)
                spnop(act_d)
                exit_prods.append(nc.sync.dma_start(out=dd[:, :], in_=dsig))

            # --- main loop: 16 groups, 16 rows each ---
            last_gen = last_mm = last_act = None
            vq = []
            for c in range(4):
                vq_t = vout.tile([128, QW[c]], bf16, tag=f"vq{c}", name=f"vq{c}")
                vq.append(vq_t)
            qoff = [0, 0, 0, 0]
            for g in range(NG):
                w = WIDTHS[g]
                joff = 32 * g
                c = g % 4
                ps = acc_ps.tile([128, w], fp32, tag="ps")
                for k in range(16):
                    p = 16 * g + k          # processing position (abias col)
                    eng = GEN_ENG[k]
                    if eng == 'D':
                        m = mdve.tile([H, w], bf16, tag="mdve")
                        last_gen = nc.vector.tensor_scalar(
                            out=m, in0=c2[:, joff:joff + w],
                            scalar1=abias[:, p:p + 1], scalar2=0.0,
                            op0=mybir.AluOpType.add, op1=mybir.AluOpType.max)
                    elif eng == 'P':
                        m = mpool.tile([H, w], bf16, tag="mpool")
                        last_gen = nc.gpsimd.tensor_scalar(
                            out=m, in0=c2[:, joff:joff + w],
                            scalar1=abias[:, p:p + 1], scalar2=0.0,
                            op0=mybir.AluOpType.add, op1=mybir.AluOpType.max)
                    else:
                        m = mact.tile([H, w], bf16, tag="mact")
                        last_gen = nc.scalar.activation(
                            out=m, in_=c2[:, joff:joff + w],
                            func=mybir.ActivationFunctionType.Relu,
                            bias=abias[:, p:p + 1], scale=1.0)
                    last_mm = nc.tensor.matmul(
                        ps[32 * c:32 * c + 32, :],
                        zz[:, 32 - k:64 - k], m,
                        start=(k == 0), stop=(k == 15),
                        tile_position=(0, 32 * c))
                o = qoff[c]
                last_act = nc.scalar.activation(
                    out=vq[c][32 * c:32 * c + 16, o:o + w],
                    in_=ps[32 * c:32 * c + 16, :],
                    func=mybir.ActivationFunctionType.Sigmoid,
                    bias=b2c[32 * c:32 * c + 16, :], scale=1.0)
                qoff[c] = o + w
                if g == 0:
                    emit_diag()
                if g >= 12:  # quadrant complete after its 4th group
                    spnop(last_act)
                    exit_prods.append(
                        nc.sync.dma_start(out=outs[c][:, :],
                                          in_=vq[c][32 * c:32 * c + 16, :]))

            # exit fan-in: give the tail drain an SP-observed clock
            for p in exit_prods:
                if p is not None:
                    spnop(p)
    return nc


def _split_waits(bir_bytes):
    """Post-pass: walrus on this image accepts ~1 sem wait per instruction.
    Hoist all-but-one wait of any multi-wait instruction onto same-engine
    NoOps inserted immediately before it (engine stalls on each in order --
    semantically identical, within the 1-wait limit)."""
    import json
    bir = json.loads(bir_bytes)
    counter = [0]

    def mknop(engine, wait, debug):
        counter[0] += 1
        return {
            "debug": debug,
            "engine": engine,
            "ins": [],
            "name": f"WSN-{counter[0]}",
            "opcode": "NoOp",
            "outs": [],
            "sync_info": {"on_update": [], "on_wait": [wait]},
        }

    def process(blocks):
        for blk in blocks:
            insts = blk.get("instructions")
            if not insts:
                continue
            out = []
            for ins in insts:
                si = ins.get("sync_info")
                ow = (si or {}).get("on_wait") or []
                if len(ow) > 1:
                    for w in ow[:-1]:
                        out.append(mknop(ins["engine"], w, ins.get("debug", 0)))
                    si["on_wait"] = [ow[-1]]
                out.append(ins)
            blk["instructions"] = out

    for func in bir.get("functions", []):
        process(func.get("blocks", []))
    return json.dumps(bir).encode()


def _get_nc():
    with _lock:
        if "nc" not in _cache:
            nc = _build_nc()
            orig = nc.to_json_bytes
            nc.to_json_bytes = lambda: _split_waits(orig())
            _cache["nc"] = nc
        return _cache["nc"]


def make_core_inputs(X, W1c, b1c, W2c, b2c, W1s, b1s, W2s, b2s):
    """Build the 8 per-core input maps (host-side weight folding)."""
    X = np.ascontiguousarray(np.asarray(X, np.float32))
    w2c = np.asarray(W2c, np.float32).reshape(-1)
    w2s = np.asarray(W2s, np.float32).reshape(-1)
    aw, sg = np.abs(w2c), np.sign(w2c).astype(np.float32)
    aws, sgs = np.abs(w2s), np.sign(w2s).astype(np.float32)
    W1c = np.asarray(W1c, np.float32)

    base16 = np.zeros((128, PK16_W), np.float32)
    base16[:, C_W1A:C_W1A + 128] = W1c[:D] * aw[None, :]
    base16[:, C_W1B:C_W1B + 128] = W1c[D:] * aw[None, :]
    base16[:, C_W1S:C_W1S + 128] = np.asarray(W1s, np.float32) * aws[None, :]
    base16[:, C_ZZ + 32] = sg
    base16[:, C_ZS] = sgs

    p32 = np.zeros((128, PK32_W), np.float32)
    p32[:, 0] = np.asarray(b1c, np.float32) * aw
    p32[:, 1] = np.asarray(b1s, np.float32) * aws
    p32[:, 2] = float(np.asarray(b2c).reshape(-1)[0])
    p32[:, 3] = float(np.asarray(b2s).reshape(-1)[0])

    in_maps = []
    for core in range(NCORES):
        b, r = core // 2, core % 2
        rows = np.concatenate([
            np.arange(32 * g + 16 * r, 32 * g + 16 * r + 16) for g in range(NG)])
        p16 = base16.copy()
        p16[:, C_XT:C_XT + 512] = X[b].T
        p16[:, C_XIT:C_XIT + 256] = X[b, rows].T
        in_maps.append({"pk32": p32,
                        "pk16": p16.astype(ml_dtypes.bfloat16)})
    return in_maps


def assemble(results, dtype=np.float32):
    """results: list of 8 dicts with q0..q3 [16, QW_c] bf16, dd [1,256] f32.
    Quadrant c packs groups g = c, c+4, c+8, c+12 at increasing col offsets."""
    sim = np.zeros((B, N, N), np.float32)
    for b in range(B):
        U = np.zeros((N, N), np.float32)
        diag = np.zeros(N, np.float32)
        for r in range(2):
            res = results[2 * b + r]
            d = np.asarray(res["dd"]).reshape(256)
            qoff = [0, 0, 0, 0]
            for g in range(NG):
                c, w = g % 4, WIDTHS[g]
                rows = slice(32 * g + 16 * r, 32 * g + 16 * r + 16)
                o = qoff[c]
                U[rows, 32 * g:] = np.asarray(
                    res[f"q{c}"][:, o:o + w], np.float32)
                qoff[c] = o + w
                diag[rows] = d[16 * g:16 * g + 16]
        Ut = np.triu(U, 1)
        out = Ut + Ut.T
        np.fill_diagonal(out, diag)
        sim[b] = out
    return sim.astype(dtype)


def kernel(X, W1c, b1c, W2c, b2c, W1s, b1s, W2s, b2s):
    from concourse.bass_utils import run_bass_kernel_spmd

    nc = _get_nc()
    in_maps = make_core_inputs(X, W1c, b1c, W2c, b2c, W1s, b1s, W2s, b2s)
    res = run_bass_kernel_spmd(nc, in_maps, core_ids=list(range(NCORES)))
    return assemble(res.results, dtype=np.asarray(X).dtype)


# revision 15
# speedup vs baseline: 1.0358x; 1.0358x over previous
"""ClusterMergeNet Trainium2 kernel, v2: triangle-hugging tiling.

Math: sim[b,i,j] (i<j) = sigmoid( sum_h W2c[h]*relu(A[b,i,h] + C[b,j,h] + b1c[h]) + b2c )
  with A = X @ W1c[:D], C = X @ W1c[D:]   (pair-MLP first layer decomposed).
Diagonal: sigmoid( sum_h W2s[h]*relu((X@W1s)[b,i,h] + b1s[h]) + b2s ).

|W2c| is folded into the first-layer weights/bias on the host so the device
reduction over h is a signed sum via one-hot sign-column matmuls on the PE.

Tiling (v2): per batch b, rows are split into 16 groups of 32 (group g =
rows [32g, 32g+32)); the upper triangle only needs j >= 32g for group g, so
the j-span is [32g, 512) (width 512-32g).  Core role r in {0,1} (cores 2b+r)
takes the 16-row half [32g+16r, 32g+16(r+1)) of every group -> all 8 cores
run an IDENTICAL instruction stream (SPMD); only the packed xit data differs.
Work per core: 16 groups x 16 rows x (512-32g) cols = 69632 streamed columns
vs 98304 in v1 (-29%).

Generation m = relu(C_tile + a_i) is split over three engines (DVE fastest
with the 2-byte 4x mode, then ACT, then Pool at 0.6 efficiency); the PE
consumes m tiles with sliding sign-window matmuls accumulating 16 rows into
one PSUM quadrant per group; ACT applies sigmoid and DMAs [16, w] bf16 tiles.

The walrus build on this image supports only ~1 semaphore wait per hardware
instruction; same mitigation as v1 (packed input DMAs + per-engine fan-in
NOPs, per-engine tile pools, NOP-guarded output DMAs, _split_waits postpass).
"""

import threading

import numpy as np
import ml_dtypes

B, N, D, H = 4, 512, 128, 128
NCORES = 8
NG = 16                      # groups per core
WIDTHS = [512 - 32 * g for g in range(NG)]
ORDER = [12, 1, 2, 3, 0, 5, 6, 7, 4, 9, 10, 11, 8, 13, 14, 15]

# bf16 pack layout (columns)
C_XT = 0            # [0,512)    X_b.T
C_XIT = 512         # [512,768)  own 256 i-rows (processing order), transposed
C_W1A = 768         # [768,896)  W1c[:D]*|W2c|
C_W1B = 896         # [896,1024) W1c[D:]*|W2c|
C_W1S = 1024        # [1024,1152) W1s*|W2s|
C_ZZ = 1152         # [1152,1216) zz (col C_ZZ+32 = sign(W2c))
C_ZS = 1216         # sign(W2s)
PK16_W = 1217
# fp32 pack: beffc, b1se, b2c, b2s
PK32_W = 4

# engine per within-group position k: 10x DVE, 2x ACT, 4x Pool (interleaved)
GEN_ENG = ['D', 'D', 'P', 'D', 'A', 'D', 'P', 'D',
           'D', 'D', 'P', 'D', 'A', 'D', 'D', 'D']

_lock = threading.Lock()
_cache = {}


def _build_nc():
    import concourse.bass as bass
    import concourse.mybir as mybir
    import concourse.tile as tile
    from concourse.tile import add_dep_helper

    fp32 = mybir.dt.float32
    bf16 = mybir.dt.bfloat16

    nc = bass.Bass("TRN2")
    pk32 = nc.dram_tensor("pk32", [128, PK32_W], fp32, kind="ExternalInput")
    pk16 = nc.dram_tensor("pk16", [128, PK16_W], bf16, kind="ExternalInput")
    QW = [sum(WIDTHS[c + 4 * t] for t in range(4)) for c in range(4)]
    outs = [nc.dram_tensor(f"q{c}", [16, QW[c]], bf16, kind="ExternalOutput")
            for c in range(4)]
    dd = nc.dram_tensor("dd", [1, 256], fp32, kind="ExternalOutput")

    exit_prods = []

    def spnop(prod):
        n = nc.sync.nop(nofuse=True)
        add_dep_helper(n.ins, prod.ins, sync=True, reason="sp fanin")
        return n

    with tile.TileContext(nc) as tc:
        with (
            tc.tile_pool(name="singles", bufs=1) as singles,
            tc.tile_pool(name="mdve", bufs=28) as mdve,
            tc.tile_pool(name="mact", bufs=6) as mact,
            tc.tile_pool(name="mpool", bufs=12) as mpool,
            tc.tile_pool(name="vout", bufs=1) as vout,
            tc.tile_pool(name="pre_ps", bufs=1, space="PSUM") as pre_ps,
            tc.tile_pool(name="acc_ps", bufs=6, space="PSUM") as acc_ps,
        ):
            p16 = singles.tile([128, PK16_W], bf16, tag="p16")
            dma16 = nc.sync.dma_start(out=p16, in_=pk16[:, :])
            p32 = singles.tile([128, PK32_W], fp32, tag="p32")
            dma32 = nc.sync.dma_start(out=p32, in_=pk32[:, :])

            xt = p16[:, C_XT:C_XT + 512]
            xit = p16[:, C_XIT:C_XIT + 256]
            w1a = p16[:, C_W1A:C_W1A + 128]
            w1b = p16[:, C_W1B:C_W1B + 128]
            w1s = p16[:, C_W1S:C_W1S + 128]
            zz = p16[:, C_ZZ:C_ZZ + 64]
            zs = p16[:, C_ZS:C_ZS + 1]
            beffc = p32[:, 0:1]
            b1se = p32[:, 1:2]
            b2c = p32[:, 2:3]
            b2s = p32[:, 3:4]

            # --- precompute: abias = (w1a.T @ xit) + beffc   [H, 256] f32 ---
            apsum = pre_ps.tile([H, 256], fp32, tag="pre")
            nc.tensor.matmul(apsum, w1a, xit, start=True, stop=True)
            abias = singles.tile([H, 256], fp32, tag="abias")
            nc.vector.tensor_scalar(out=abias, in0=apsum, scalar1=beffc,
                                    scalar2=None, op0=mybir.AluOpType.add)

            # --- c2 = (w1b.T @ xt)  [H, N] bf16 (bias lives in abias) ---
            cpsum = pre_ps.tile([H, N], fp32, tag="cpsum")
            nc.tensor.matmul(cpsum, w1b, xt, start=True, stop=True)
            c2 = singles.tile([H, N], bf16, tag="c2")
            nc.vector.tensor_copy(out=c2, in_=cpsum)

            spsum = pre_ps.tile([H, 256], fp32, tag="pre", name="spsum")
            nc.tensor.matmul(spsum, w1s, xit, start=True, stop=True)

            def emit_diag():
                # rest of the self-sim diagonal (emitted after group 0 so it
                # stays off the prologue critical path)
                ms = singles.tile([H, 256], bf16, tag="ms")
                nc.vector.tensor_scalar(out=ms, in0=spsum, scalar1=b1se,
                                        scalar2=0.0, op0=mybir.AluOpType.add,
                                        op1=mybir.AluOpType.max)
                dpsum = pre_ps.tile([128, 256], fp32, tag="pre", name="dpsum")
                nc.tensor.matmul(dpsum[0:1, :], zs, ms, start=True, stop=True)
                dsig = singles.tile([1, 256], fp32, tag="dsig")
                act_d = nc.scalar.activation(
                    out=dsig, in_=dpsum[0:1, :],
                    func=mybir.ActivationFunctionType.Sigmoid,
                    bias=b2s[0:1, :], scale=1.0)
                spnop(act_d)
                exit_prods.append(nc.sync.dma_start(out=dd[:, :], in_=dsig))

            # --- main loop: 16 groups, 16 rows each ---
            last_gen = last_mm = last_act = None
            vq = []
            for c in range(4):
                vq_t = vout.tile([128, QW[c]], bf16, tag=f"vq{c}", name=f"vq{c}")
                vq.append(vq_t)
            qoff = [0, 0, 0, 0]
            qcnt = [0, 0, 0, 0]
            for g in ORDER:
                w = WIDTHS[g]
                joff = 32 * g
                c = g % 4
                ps = acc_ps.tile([128, w], fp32, tag="ps")
                for k in range(16):
                    p = 16 * g + k          # processing position (abias col)
                    eng = GEN_ENG[k]
                    if eng == 'D':
                        m = mdve.tile([H, w], bf16, tag="mdve")
                        last_gen = nc.vector.tensor_scalar(
                            out=m, in0=c2[:, joff:joff + w],
                            scalar1=abias[:, p:p + 1], scalar2=0.0,
                            op0=mybir.AluOpType.add, op1=mybir.AluOpType.max)
                    elif eng == 'P':
                        m = mpool.tile([H, w], bf16, tag="mpool")
                        last_gen = nc.gpsimd.tensor_scalar(
                            out=m, in0=c2[:, joff:joff + w],
                            scalar1=abias[:, p:p + 1], scalar2=0.0,
                            op0=mybir.AluOpType.add, op1=mybir.AluOpType.max)
                    else:
                        m = mact.tile([H, w], bf16, tag="mact")
                        last_gen = nc.scalar.activation(
                            out=m, in_=c2[:, joff:joff + w],
                            func=mybir.ActivationFunctionType.Relu,
                            bias=abias[:, p:p + 1], scale=1.0)
                    last_mm = nc.tensor.matmul(
                        ps[32 * c:32 * c + 32, :],
                        zz[:, 32 - k:64 - k], m,
                        start=(k == 0), stop=(k == 15),
                        tile_position=(0, 32 * c))
                o = qoff[c]
                last_act = nc.scalar.activation(
                    out=vq[c][32 * c:32 * c + 16, o:o + w],
                    in_=ps[32 * c:32 * c + 16, :],
                    func=mybir.ActivationFunctionType.Sigmoid,
                    bias=b2c[32 * c:32 * c + 16, :], scale=1.0)
                qoff[c] = o + w
                if g == 0:
                    emit_diag()
                qcnt[c] += 1
                if qcnt[c] == 4:  # quadrant complete after its 4th group
                    spnop(last_act)
                    exit_prods.append(
                        nc.sync.dma_start(out=outs[c][:, :],
                                          in_=vq[c][32 * c:32 * c + 16, :]))

            # exit fan-in: give the tail drain an SP-observed clock
            for p in exit_prods:
                if p is not None:
                    spnop(p)
    return nc


def _split_waits(bir_bytes):
    """Post-pass: walrus on this image accepts ~1 sem wait per instruction.
    Hoist all-but-one wait of any multi-wait instruction onto same-engine
    NoOps inserted immediately before it (engine stalls on each in order --
    semantically identical, within the 1-wait limit)."""
    import json
    bir = json.loads(bir_bytes)
    counter = [0]

    def mknop(engine, wait, debug):
        counter[0] += 1
        return {
            "debug": debug,
            "engine": engine,
            "ins": [],
            "name": f"WSN-{counter[0]}",
            "opcode": "NoOp",
            "outs": [],
            "sync_info": {"on_update": [], "on_wait": [wait]},
        }

    def process(blocks):
        for blk in blocks:
            insts = blk.get("instructions")
            if not insts:
                continue
            out = []
            for ins in insts:
                si = ins.get("sync_info")
                ow = (si or {}).get("on_wait") or []
                if len(ow) > 1:
                    for w in ow[:-1]:
                        out.append(mknop(ins["engine"], w, ins.get("debug", 0)))
                    si["on_wait"] = [ow[-1]]
                out.append(ins)
            blk["instructions"] = out

    for func in bir.get("functions", []):
        process(func.get("blocks", []))
    return json.dumps(bir).encode()


def _get_nc():
    with _lock:
        if "nc" not in _cache:
            nc = _build_nc()
            orig = nc.to_json_bytes
            nc.to_json_bytes = lambda: _split_waits(orig())
            _cache["nc"] = nc
        return _cache["nc"]


def make_core_inputs(X, W1c, b1c, W2c, b2c, W1s, b1s, W2s, b2s):
    """Build the 8 per-core input maps (host-side weight folding)."""
    X = np.ascontiguousarray(np.asarray(X, np.float32))
    w2c = np.asarray(W2c, np.float32).reshape(-1)
    w2s = np.asarray(W2s, np.float32).reshape(-1)
    aw, sg = np.abs(w2c), np.sign(w2c).astype(np.float32)
    aws, sgs = np.abs(w2s), np.sign(w2s).astype(np.float32)
    W1c = np.asarray(W1c, np.float32)

    base16 = np.zeros((128, PK16_W), np.float32)
    base16[:, C_W1A:C_W1A + 128] = W1c[:D] * aw[None, :]
    base16[:, C_W1B:C_W1B + 128] = W1c[D:] * aw[None, :]
    base16[:, C_W1S:C_W1S + 128] = np.asarray(W1s, np.float32) * aws[None, :]
    base16[:, C_ZZ + 32] = sg
    base16[:, C_ZS] = sgs

    p32 = np.zeros((128, PK32_W), np.float32)
    p32[:, 0] = np.asarray(b1c, np.float32) * aw
    p32[:, 1] = np.asarray(b1s, np.float32) * aws
    p32[:, 2] = float(np.asarray(b2c).reshape(-1)[0])
    p32[:, 3] = float(np.asarray(b2s).reshape(-1)[0])

    in_maps = []
    for core in range(NCORES):
        b, r = core // 2, core % 2
        rows = np.concatenate([
            np.arange(32 * g + 16 * r, 32 * g + 16 * r + 16) for g in range(NG)])
        p16 = base16.copy()
        p16[:, C_XT:C_XT + 512] = X[b].T
        p16[:, C_XIT:C_XIT + 256] = X[b, rows].T
        in_maps.append({"pk32": p32,
                        "pk16": p16.astype(ml_dtypes.bfloat16)})
    return in_maps


def assemble(results, dtype=np.float32):
    """results: list of 8 dicts with q0..q3 [16, QW_c] bf16, dd [1,256] f32.
    Quadrant c packs groups g = c, c+4, c+8, c+12 at increasing col offsets."""
    sim = np.zeros((B, N, N), np.float32)
    for b in range(B):
        U = np.zeros((N, N), np.float32)
        diag = np.zeros(N, np.float32)
        for r in range(2):
            res = results[2 * b + r]
            d = np.asarray(res["dd"]).reshape(256)
            qoff = [0, 0, 0, 0]
            for g in ORDER:
                c, w = g % 4, WIDTHS[g]
                rows = slice(32 * g + 16 * r, 32 * g + 16 * r + 16)
                o = qoff[c]
                U[rows, 32 * g:] = np.asarray(
                    res[f"q{c}"][:, o:o + w], np.float32)
                qoff[c] = o + w
                diag[rows] = d[16 * g:16 * g + 16]
        Ut = np.triu(U, 1)
        out = Ut + Ut.T
        np.fill_diagonal(out, diag)
        sim[b] = out
    return sim.astype(dtype)


def kernel(X, W1c, b1c, W2c, b2c, W1s, b1s, W2s, b2s):
    from concourse.bass_utils import run_bass_kernel_spmd

    nc = _get_nc()
    in_maps = make_core_inputs(X, W1c, b1c, W2c, b2c, W1s, b1s, W2s, b2s)
    res = run_bass_kernel_spmd(nc, in_maps, core_ids=list(range(NCORES)))
    return assemble(res.results, dtype=np.asarray(X).dtype)


# revision 19
# speedup vs baseline: 1.0439x; 1.0078x over previous
"""ClusterMergeNet Trainium2 kernel, v2: triangle-hugging tiling.

Math: sim[b,i,j] (i<j) = sigmoid( sum_h W2c[h]*relu(A[b,i,h] + C[b,j,h] + b1c[h]) + b2c )
  with A = X @ W1c[:D], C = X @ W1c[D:]   (pair-MLP first layer decomposed).
Diagonal: sigmoid( sum_h W2s[h]*relu((X@W1s)[b,i,h] + b1s[h]) + b2s ).

|W2c| is folded into the first-layer weights/bias on the host so the device
reduction over h is a signed sum via one-hot sign-column matmuls on the PE.

Tiling (v2): per batch b, rows are split into 16 groups of 32 (group g =
rows [32g, 32g+32)); the upper triangle only needs j >= 32g for group g, so
the j-span is [32g, 512) (width 512-32g).  Core role r in {0,1} (cores 2b+r)
takes the 16-row half [32g+16r, 32g+16(r+1)) of every group -> all 8 cores
run an IDENTICAL instruction stream (SPMD); only the packed xit data differs.
Work per core: 16 groups x 16 rows x (512-32g) cols = 69632 streamed columns
vs 98304 in v1 (-29%).

Generation m = relu(C_tile + a_i) is split over three engines (DVE fastest
with the 2-byte 4x mode, then ACT, then Pool at 0.6 efficiency); the PE
consumes m tiles with sliding sign-window matmuls accumulating 16 rows into
one PSUM quadrant per group; ACT applies sigmoid and DMAs [16, w] bf16 tiles.

The walrus build on this image supports only ~1 semaphore wait per hardware
instruction; same mitigation as v1 (packed input DMAs + per-engine fan-in
NOPs, per-engine tile pools, NOP-guarded output DMAs, _split_waits postpass).
"""

import threading

import numpy as np
import ml_dtypes

B, N, D, H = 4, 512, 128, 128
NCORES = 8
NG = 16                      # groups per core
WIDTHS = [512 - 32 * g for g in range(NG)]
ORDER = [12, 13, 2, 3, 0, 1, 6, 7, 4, 5, 10, 11, 8, 9, 14, 15]

# bf16 pack layout (columns)
C_XT = 0            # [0,512)    X_b.T
C_XIT = 512         # [512,768)  own 256 i-rows (processing order), transposed
C_W1A = 768         # [768,896)  W1c[:D]*|W2c|
C_W1B = 896         # [896,1024) W1c[D:]*|W2c|
C_W1S = 1024        # [1024,1152) W1s*|W2s|
C_ZZ = 1152         # [1152,1216) zz (col C_ZZ+32 = sign(W2c))
C_ZS = 1216         # sign(W2s)
PK16_W = 1217
# fp32 pack: beffc, b1se, b2c, b2s
PK32_W = 4

# engine per within-group position k: 10x DVE, 2x ACT, 4x Pool (interleaved)
GEN_ENG = ['D', 'D', 'P', 'D', 'A', 'D', 'P', 'D',
           'D', 'D', 'P', 'D', 'A', 'D', 'D', 'D']

_lock = threading.Lock()
_cache = {}


def _build_nc():
    import concourse.bass as bass
    import concourse.mybir as mybir
    import concourse.tile as tile
    from concourse.tile import add_dep_helper

    fp32 = mybir.dt.float32
    bf16 = mybir.dt.bfloat16

    nc = bass.Bass("TRN2")
    pk32 = nc.dram_tensor("pk32", [128, PK32_W], fp32, kind="ExternalInput")
    pk16 = nc.dram_tensor("pk16", [128, PK16_W], bf16, kind="ExternalInput")
    QW = [sum(WIDTHS[c + 4 * t] for t in range(4)) for c in range(4)]
    outs = [nc.dram_tensor(f"q{c}", [16, QW[c]], bf16, kind="ExternalOutput")
            for c in range(4)]
    dd = nc.dram_tensor("dd", [1, 256], fp32, kind="ExternalOutput")

    exit_prods = []

    def spnop(prod):
        n = nc.sync.nop(nofuse=True)
        add_dep_helper(n.ins, prod.ins, sync=True, reason="sp fanin")
        return n

    with tile.TileContext(nc) as tc:
        with (
            tc.tile_pool(name="singles", bufs=1) as singles,
            tc.tile_pool(name="mdve", bufs=28) as mdve,
            tc.tile_pool(name="mact", bufs=6) as mact,
            tc.tile_pool(name="mpool", bufs=12) as mpool,
            tc.tile_pool(name="vout", bufs=1) as vout,
            tc.tile_pool(name="pre_ps", bufs=1, space="PSUM") as pre_ps,
            tc.tile_pool(name="acc_ps", bufs=6, space="PSUM") as acc_ps,
        ):
            p16 = singles.tile([128, PK16_W], bf16, tag="p16")
            dma16 = nc.sync.dma_start(out=p16, in_=pk16[:, :])
            p32 = singles.tile([128, PK32_W], fp32, tag="p32")
            dma32 = nc.sync.dma_start(out=p32, in_=pk32[:, :])

            xt = p16[:, C_XT:C_XT + 512]
            xit = p16[:, C_XIT:C_XIT + 256]
            w1a = p16[:, C_W1A:C_W1A + 128]
            w1b = p16[:, C_W1B:C_W1B + 128]
            w1s = p16[:, C_W1S:C_W1S + 128]
            zz = p16[:, C_ZZ:C_ZZ + 64]
            zs = p16[:, C_ZS:C_ZS + 1]
            beffc = p32[:, 0:1]
            b1se = p32[:, 1:2]
            b2c = p32[:, 2:3]
            b2s = p32[:, 3:4]

            # --- precompute: abias = (w1a.T @ xit) + beffc   [H, 256] f32 ---
            apsum = pre_ps.tile([H, 256], fp32, tag="pre")
            nc.tensor.matmul(apsum, w1a, xit, start=True, stop=True)
            abias = singles.tile([H, 256], fp32, tag="abias")
            nc.vector.tensor_scalar(out=abias, in0=apsum, scalar1=beffc,
                                    scalar2=None, op0=mybir.AluOpType.add)

            # --- c2 = (w1b.T @ xt)  [H, N] bf16 (bias lives in abias) ---
            cpsum = pre_ps.tile([H, N], fp32, tag="cpsum")
            nc.tensor.matmul(cpsum, w1b, xt, start=True, stop=True)
            c2 = singles.tile([H, N], bf16, tag="c2")
            nc.vector.tensor_copy(out=c2, in_=cpsum)

            spsum = pre_ps.tile([H, 256], fp32, tag="pre", name="spsum")
            nc.tensor.matmul(spsum, w1s, xit, start=True, stop=True)

            def emit_diag():
                # rest of the self-sim diagonal (emitted after group 0 so it
                # stays off the prologue critical path)
                ms = singles.tile([H, 256], bf16, tag="ms")
                nc.vector.tensor_scalar(out=ms, in0=spsum, scalar1=b1se,
                                        scalar2=0.0, op0=mybir.AluOpType.add,
                                        op1=mybir.AluOpType.max)
                dpsum = pre_ps.tile([128, 256], fp32, tag="pre", name="dpsum")
                nc.tensor.matmul(dpsum[0:1, :], zs, ms, start=True, stop=True)
                dsig = singles.tile([1, 256], fp32, tag="dsig")
                act_d = nc.scalar.activation(
                    out=dsig, in_=dpsum[0:1, :],
                    func=mybir.ActivationFunctionType.Sigmoid,
                    bias=b2s[0:1, :], scale=1.0)
                spnop(act_d)
                exit_prods.append(nc.sync.dma_start(out=dd[:, :], in_=dsig))

            # --- main loop: 16 groups, 16 rows each ---
            last_gen = last_mm = last_act = None
            vq = []
            for c in range(4):
                vq_t = vout.tile([128, QW[c]], bf16, tag=f"vq{c}", name=f"vq{c}")
                vq.append(vq_t)
            qoff = [0, 0, 0, 0]
            qcnt = [0, 0, 0, 0]
            for g in ORDER:
                w = WIDTHS[g]
                joff = 32 * g
                c = g % 4
                ps = acc_ps.tile([128, w], fp32, tag="ps")
                for k in range(16):
                    p = 16 * g + k          # processing position (abias col)
                    eng = GEN_ENG[k]
                    if eng == 'D':
                        m = mdve.tile([H, w], bf16, tag="mdve")
                        last_gen = nc.vector.tensor_scalar(
                            out=m, in0=c2[:, joff:joff + w],
                            scalar1=abias[:, p:p + 1], scalar2=0.0,
                            op0=mybir.AluOpType.add, op1=mybir.AluOpType.max)
                    elif eng == 'P':
                        m = mpool.tile([H, w], bf16, tag="mpool")
                        last_gen = nc.gpsimd.tensor_scalar(
                            out=m, in0=c2[:, joff:joff + w],
                            scalar1=abias[:, p:p + 1], scalar2=0.0,
                            op0=mybir.AluOpType.add, op1=mybir.AluOpType.max)
                    else:
                        m = mact.tile([H, w], bf16, tag="mact")
                        last_gen = nc.scalar.activation(
                            out=m, in_=c2[:, joff:joff + w],
                            func=mybir.ActivationFunctionType.Relu,
                            bias=abias[:, p:p + 1], scale=1.0)
                    last_mm = nc.tensor.matmul(
                        ps[32 * c:32 * c + 32, :],
                        zz[:, 32 - k:64 - k], m,
                        start=(k == 0), stop=(k == 15),
                        tile_position=(0, 32 * c))
                o = qoff[c]
                last_act = nc.scalar.activation(
                    out=vq[c][32 * c:32 * c + 16, o:o + w],
                    in_=ps[32 * c:32 * c + 16, :],
                    func=mybir.ActivationFunctionType.Sigmoid,
                    bias=b2c[32 * c:32 * c + 16, :], scale=1.0)
                qoff[c] = o + w
                if g == 0:
                    emit_diag()
                qcnt[c] += 1
                if qcnt[c] == 4:  # quadrant complete after its 4th group
                    spnop(last_act)
                    exit_prods.append(
                        nc.sync.dma_start(out=outs[c][:, :],
                                          in_=vq[c][32 * c:32 * c + 16, :]))

            # exit fan-in: give the tail drain an SP-observed clock
            for p in exit_prods:
                if p is not None:
                    spnop(p)
    return nc


def _split_waits(bir_bytes):
    """Post-pass: walrus on this image accepts ~1 sem wait per instruction.
    Hoist all-but-one wait of any multi-wait instruction onto same-engine
    NoOps inserted immediately before it (engine stalls on each in order --
    semantically identical, within the 1-wait limit)."""
    import json
    bir = json.loads(bir_bytes)
    counter = [0]

    def mknop(engine, wait, debug):
        counter[0] += 1
        return {
            "debug": debug,
            "engine": engine,
            "ins": [],
            "name": f"WSN-{counter[0]}",
            "opcode": "NoOp",
            "outs": [],
            "sync_info": {"on_update": [], "on_wait": [wait]},
        }

    def process(blocks):
        for blk in blocks:
            insts = blk.get("instructions")
            if not insts:
                continue
            out = []
            for ins in insts:
                si = ins.get("sync_info")
                ow = (si or {}).get("on_wait") or []
                if len(ow) > 1:
                    for w in ow[:-1]:
                        out.append(mknop(ins["engine"], w, ins.get("debug", 0)))
                    si["on_wait"] = [ow[-1]]
                out.append(ins)
            blk["instructions"] = out

    for func in bir.get("functions", []):
        process(func.get("blocks", []))
    return json.dumps(bir).encode()


def _get_nc():
    with _lock:
        if "nc" not in _cache:
            nc = _build_nc()
            orig = nc.to_json_bytes
            nc.to_json_bytes = lambda: _split_waits(orig())
            _cache["nc"] = nc
        return _cache["nc"]


def make_core_inputs(X, W1c, b1c, W2c, b2c, W1s, b1s, W2s, b2s):
    """Build the 8 per-core input maps (host-side weight folding)."""
    X = np.ascontiguousarray(np.asarray(X, np.float32))
    w2c = np.asarray(W2c, np.float32).reshape(-1)
    w2s = np.asarray(W2s, np.float32).reshape(-1)
    aw, sg = np.abs(w2c), np.sign(w2c).astype(np.float32)
    aws, sgs = np.abs(w2s), np.sign(w2s).astype(np.float32)
    W1c = np.asarray(W1c, np.float32)

    base16 = np.zeros((128, PK16_W), np.float32)
    base16[:, C_W1A:C_W1A + 128] = W1c[:D] * aw[None, :]
    base16[:, C_W1B:C_W1B + 128] = W1c[D:] * aw[None, :]
    base16[:, C_W1S:C_W1S + 128] = np.asarray(W1s, np.float32) * aws[None, :]
    base16[:, C_ZZ + 32] = sg
    base16[:, C_ZS] = sgs

    p32 = np.zeros((128, PK32_W), np.float32)
    p32[:, 0] = np.asarray(b1c, np.float32) * aw
    p32[:, 1] = np.asarray(b1s, np.float32) * aws
    p32[:, 2] = float(np.asarray(b2c).reshape(-1)[0])
    p32[:, 3] = float(np.asarray(b2s).reshape(-1)[0])

    in_maps = []
    for core in range(NCORES):
        b, r = core // 2, core % 2
        rows = np.concatenate([
            np.arange(32 * g + 16 * r, 32 * g + 16 * r + 16) for g in range(NG)])
        p16 = base16.copy()
        p16[:, C_XT:C_XT + 512] = X[b].T
        p16[:, C_XIT:C_XIT + 256] = X[b, rows].T
        in_maps.append({"pk32": p32,
                        "pk16": p16.astype(ml_dtypes.bfloat16)})
    return in_maps


def assemble(results, dtype=np.float32):
    """results: list of 8 dicts with q0..q3 [16, QW_c] bf16, dd [1,256] f32.
    Quadrant c packs groups g = c, c+4, c+8, c+12 at increasing col offsets."""
    sim = np.zeros((B, N, N), np.float32)
    for b in range(B):
        U = np.zeros((N, N), np.float32)
        diag = np.zeros(N, np.float32)
        for r in range(2):
            res = results[2 * b + r]
            d = np.asarray(res["dd"]).reshape(256)
            qoff = [0, 0, 0, 0]
            for g in ORDER:
                c, w = g % 4, WIDTHS[g]
                rows = slice(32 * g + 16 * r, 32 * g + 16 * r + 16)
                o = qoff[c]
                U[rows, 32 * g:] = np.asarray(
                    res[f"q{c}"][:, o:o + w], np.float32)
                qoff[c] = o + w
                diag[rows] = d[16 * g:16 * g + 16]
        Ut = np.triu(U, 1)
        out = Ut + Ut.T
        np.fill_diagonal(out, diag)
        sim[b] = out
    return sim.astype(dtype)


def kernel(X, W1c, b1c, W2c, b2c, W1s, b1s, W2s, b2s):
    from concourse.bass_utils import run_bass_kernel_spmd

    nc = _get_nc()
    in_maps = make_core_inputs(X, W1c, b1c, W2c, b2c, W1s, b1s, W2s, b2s)
    res = run_bass_kernel_spmd(nc, in_maps, core_ids=list(range(NCORES)))
    return assemble(res.results, dtype=np.asarray(X).dtype)
